# revision 14
# baseline (speedup 1.0000x reference)
"""Trainium2 Bass kernel for nn_ASS_JRG_3573412790879 (gnn_message_passing).

Pure data parallelism over batch B=16 across 8 cores (2 samples/core).

v2 strategy (vs the 23us baseline):
  - All heavy matmuls in fp8 e4m3 with the DoubleRow perf mode (0.5 cyc/row):
    encoder matmuls pair the two D/2=100 k-chunks as the two k-tiles; the
    graph contraction pairs the lo (t'0..6) / hi (t'6..11) row blocks via a
    |Glo|Z|Ghi| strip layout whose single shared zero block serves both
    k-tile access patterns.
  - Per-(ty,idx) encoder bias is folded into the graph matmul: the P tile
    carries DMA'd bias rows (p119 for the lo k-tile, p127 for hi) and the G
    strips carry matching ones-rows, so every relu+accumulate op is
    bias-free and can run on either ACT or DVE with any column chunking.
  - Whole-level encoders run in the [h1, t] orientation (lhsT = W block,
    rhs = feat columns, N=12) with bias folded via the fpt ones-row; the
    J=17 broadcast weight is applied in the combine step.
  - relu+bias+sum-over-(T,slots,J) is a per-bank fused op: ACT
    activation(Relu, accum_out) / DVE tensor_scalar(max, accum_out), spread
    across both engines; the 1/(15*T*J) mean and the concat-duplication are
    folded into the regression weight on the host.
"""
import numpy as np
import ml_dtypes
from contextlib import ExitStack

import concourse.bass as bass
import concourse.bacc as bacc
import concourse.tile as tile
from concourse import mybir
from concourse.bass_utils import run_bass_kernel_spmd
from bass_rust import VecI64Pair

J, T, D, H, H1 = 17, 12, 400, 4, 128
E4 = mybir.dt.float8e4
F32 = mybir.dt.float32
BF16 = mybir.dt.bfloat16
NPE4 = ml_dtypes.float8_e4m3
NPBF = ml_dtypes.bfloat16

# G strip geometry: per ty, pair-columns = [lo cols | hi cols (+pad)]
A0, C0 = 510, 514          # ty0: 5 maps * 102, hi padded +4 to make 1024
A1, C1 = 408, 408          # ty1/ty2: 4 maps * 102
STRIPS = [(A0, C0, max(A0, C0)), (A1, C1, max(A1, C1)), (A1, C1, max(A1, C1))]
NT = [A + C for A, C, B in STRIPS]       # 1024, 816, 816 pair-cols per ty
GCOL0 = [0, 1024, 1840]                  # gtile col base per ty
NG = 2656                                # graph cols per unit (incl 4 pad)
WCOL0 = 2656                             # whole block cols in gtile
# (ty, chunk_start, chunk_end, gtile_col_start) — each within one PSUM bank
GCHUNKS = [(0, 0, 512, 0), (0, 512, 1024, 512),
           (1, 0, 512, 1024), (1, 512, 816, 1536),
           (2, 0, 208, 1840), (2, 208, 720, 2048), (2, 720, 816, 2560)]
# relu chunk list (col ranges of gtile) + engine per unit: 'A'=ACT, 'V'=DVE
RELU_CHUNKS = [(0, 1536, 'A'), (1536, 2656, 'V')]
COPY_ENG = ['A', 'V', 'A', 'V']          # per unit P-copy engine


# ---------------------------------------------------------------- host side
def _q8(x):
    return np.asarray(x, np.float32).astype(NPE4)


def _graph_mats(inputs):
    jg = np.asarray(inputs['joint_graphs'], np.float64)
    sg = np.abs(np.asarray(inputs['gs_mats'], np.float64) * jg)
    tg = np.abs(np.asarray(inputs['gt_mats'], np.float64) * jg)
    mw_s = sg * np.asarray(inputs['s_jcw'], np.float64)[:, None, :, 0]
    mw_t = tg * np.asarray(inputs['t_jcw'], np.float64)[:, None, :, 0]
    return sg, mw_s, mw_t


def _build_strips(inputs):
    """G strips [128, A+B+C] fp8 per ty; row p = t'*17+j (lo) / (t'-6)*17+j
    (hi), ones bias row at p119 (lo) / p127 (hi)."""
    sg, mw_s, mw_t = _graph_mats(inputs)
    rs_s, rs_t = mw_s.sum(2), mw_t.sum(2)
    strips = []
    for ty, (A, C, B) in enumerate(STRIPS):
        glo = np.zeros((128, A)); ghi = np.zeros((128, C))
        for half, g in ((0, glo), (1, ghi)):
            t0 = 6 * half
            for tl in range(6):
                t = t0 + tl
                for ii in range(J):
                    if ty == 0:
                        c = ii * 6 + tl
                        g[(t - t0) * J + ii, c] = 1.0           # ec0 identity
                        for h in range(H):
                            c = ((1 + h) * J + ii) * 6 + tl
                            for k in range(J):
                                g[(t - t0) * J + k, c] = sg[h, k, ii]
                    elif ty == 1:
                        for h in range(H):
                            c = (h * J + ii) * 6 + tl
                            for jj in range(J):
                                v = mw_s[h, ii, jj] - (rs_s[h, ii] if jj == ii else 0.0)
                                g[(t - t0) * J + jj, c] = v
                    else:
                        ts = min(t + 1, T - 1)
                        for h in range(H):
                            c = (h * J + ii) * 6 + tl
                            for jj in range(J):
                                g[(ts - t0) * J + jj, c] += mw_t[h, ii, jj]
                            g[(t - t0) * J + ii, c] += -rs_t[h, ii]
            ncols = A if half == 0 else (A if ty else A0)       # real cols
            g[119 if half == 0 else 127, 0:ncols] = 1.0         # bias ones row
        strip = np.concatenate([glo, np.zeros((128, B)), ghi], 1)
        strips.append(_q8(strip))
    return strips


def _host_constants(inputs):
    g0, g1, g2 = _build_strips(inputs)

    # wt [101, 2560]: patch rhs (idx, kt, ty, h) then whole lhsT blocks
    wt = np.zeros((101, 2560))
    Wty = [np.asarray(inputs['W_comm0'], np.float64),
           np.asarray(inputs['W_diff0'], np.float64),
           np.asarray(inputs['W_diff1'], np.float64)]
    for idx in range(2):
        for kt in range(2):
            for ty in range(3):
                cc = idx * 768 + kt * 384 + ty * 128
                wt[0:100, cc:cc + 128] = Wty[ty][idx][:, kt * 100:kt * 100 + 100].T
    Ww = [np.asarray(inputs['W_whole'], np.float64),
          np.asarray(inputs['W_diffw'], np.float64)]
    bw = [np.asarray(inputs['b_whole'], np.float64),
          np.asarray(inputs['b_diffw'], np.float64)]
    for idx in range(2):
        for w_ in range(2):
            base = 1536 + (idx * 2 + w_) * 256
            for kt in range(2):
                wt[0:100, base + kt * 128:base + kt * 128 + 128] = \
                    Ww[w_][idx][:, kt * 100:kt * 100 + 100].T
            wt[100, base:base + 128] = bw[w_][idx]

    bty = [np.asarray(inputs['b_comm0'], np.float64),
           np.asarray(inputs['b_diff0'], np.float64),
           np.asarray(inputs['b_diff1'], np.float64)]
    # P layout is ty-major, kt-minor: col = ty*256 + kt*128 + h
    pb = np.zeros((2, 9, 768))
    for idx in range(2):
        for ty in range(3):
            pb[idx, 0, ty * 256:ty * 256 + 128] = bty[ty][idx]        # p119, kt0
            pb[idx, 8, ty * 256 + 128:ty * 256 + 256] = bty[ty][idx]  # p127, kt1

    Wr = np.asarray(inputs['W_reg'], np.float64)
    wrt = ((Wr[:, :H1] + Wr[:, H1:]) / (15.0 * T * J)).T.astype(NPBF)
    return dict(g0=g0, g1=g1, g2=g2, wt=_q8(wt),
                pb0=_q8(pb[0]), pb1=_q8(pb[1]), wrt=wrt,
                bregt=np.asarray(inputs['b_reg'])[None].astype(NPBF))


def _host_fp(inputs, b0):
    """fpt [101, 2240] fp8. Per (s, idx) 560-col block:
    [lo-kt0 128 | lo-kt1 128 | hi-kt0 128 | hi-kt1 128 | fw0 12 | fw1 12 |
     fd0 12 | fd1 12]; lo = patch tj cols 0:119 (t'0..6), hi = tj 102:204
    (t'6..11), zero-padded to 128; kt = the two 100-d chunks of the idx
    half; partition 100 = ones (whole-enc bias row)."""
    fp = np.asarray(inputs['feat_patch'], np.float32)
    fw = np.asarray(inputs['feat_whole'], np.float32)
    out = np.zeros((101, 2, 2, 560), np.float32)
    for s in range(2):
        b = b0 + s
        fd = np.abs(np.concatenate([fw[b, 1:], fw[b, -1:]], 0) - fw[b])
        for idx in range(2):
            for kt in range(2):
                dsl = slice((idx * 2 + kt) * 100, (idx * 2 + kt) * 100 + 100)
                tj = fp[b, :, :, dsl].transpose(2, 1, 0).reshape(100, 204)
                out[0:100, s, idx, kt * 128:kt * 128 + 119] = tj[:, 0:119]
                out[0:100, s, idx, 256 + kt * 128:256 + kt * 128 + 102] = tj[:, 102:204]
                out[0:100, s, idx, 512 + kt * 12:512 + kt * 12 + 12] = fw[b, :, dsl].T
                out[0:100, s, idx, 536 + kt * 12:536 + kt * 12 + 12] = fd[:, dsl].T
        out[100, s, :, :] = 1.0
    return _q8(out.reshape(101, 2240))


# ------------------------------------------------------------- numpy emulator
def _emulate(inputs):
    """Mirror the device pipeline exactly (layouts + fp8 rounding) on numpy."""
    C = _host_constants(inputs)
    strips = [C['g0'].astype(np.float32), C['g1'].astype(np.float32),
              C['g2'].astype(np.float32)]
    wt = C['wt'].astype(np.float32)
    wrt = C['wrt'].astype(np.float32)
    bregt = C['bregt'].astype(np.float32)
    out_all = np.zeros((16, 1, 512), np.float32)
    for core in range(8):
        fpt = _host_fp(inputs, 2 * core).astype(np.float32)
        fpt = fpt.reshape(101, 2, 2, 560)
        M2 = np.zeros((128, 2), np.float32)
        for s in range(2):
            # whole-level: out[h, t] per (idx, w_)
            acc_w = np.zeros(128)
            for idx in range(2):
                for w_ in range(2):
                    base = 1536 + (idx * 2 + w_) * 256
                    pre = np.zeros((128, 12))
                    for kt in range(2):
                        lhsT = wt[:, base + kt * 128:base + kt * 128 + 128]
                        rhs = fpt[:, s, idx, 512 + w_ * 24 + kt * 12:
                                  512 + w_ * 24 + kt * 12 + 12]
                        pre += lhsT.T @ rhs
                    acc_w += np.maximum(pre, 0).sum(1)
            for idx in range(2):
                # patch enc -> P [128, 768] fp8, ty-major kt(half)-minor
                P = np.zeros((128, 768), np.float32)
                for half in range(2):
                    pre = np.zeros((128, 384))
                    for kt in range(2):
                        lhsT = fpt[:, s, idx, half * 256 + kt * 128:
                                  half * 256 + kt * 128 + 128]
                        rhs = wt[:, idx * 768 + kt * 384:idx * 768 + kt * 384 + 384]
                        pre += lhsT.T @ rhs
                    for ty in range(3):
                        P[0:119, ty * 256 + half * 128:ty * 256 + half * 128 + 128] = \
                            pre[0:119, ty * 128:ty * 128 + 128]
                P = _q8(P).astype(np.float32)
                P[119:128, :] = (C['pb0'] if idx == 0 else C['pb1']).astype(np.float32)
                # graph DR matmuls -> g [128, 2656]
                g = np.zeros((128, NG), np.float32)
                for ty, a, b, gc in GCHUNKS:
                    B = STRIPS[ty][2]
                    strip = strips[ty]
                    if0 = strip[:, a:b]
                    if1 = strip[:, B + a:B + b]
                    l0 = P[:, ty * 256:ty * 256 + 128]
                    l1 = P[:, ty * 256 + 128:ty * 256 + 256]
                    g[:, gc:gc + (b - a)] = l0.T @ if0 + l1.T @ if1
                M2[:, s] += np.maximum(g, 0).sum(1) + J * acc_w * (idx == 0)
        M2q = M2.astype(NPBF).astype(np.float32)
        reg = np.maximum(M2q.T @ wrt + bregt, 0)
        out_all[2 * core:2 * core + 2, 0, :] = reg
    return out_all


# ---------------------------------------------------------------- device side
DR = mybir.MatmulPerfMode.DoubleRow


def _ap3(t, base, pcount, ktstride, n):
    """3-dim AP [partition, ktile(2), n] over a 2-D tile, starting at element
    column `base`; used for DoubleRow matmul operands."""
    a = t[:].copy()
    rowstride = a.ap[0][0]
    a.ap = VecI64Pair([[rowstride, pcount], [ktstride, 2], [1, n]])
    a.offset = a.offset + base
    return a


def _apn(t, base, pcount, dims):
    """Custom free-dim AP: dims = [(stride, count), ...]."""
    a = t[:].copy()
    rowstride = a.ap[0][0]
    a.ap = VecI64Pair([[rowstride, pcount]] + [list(d) for d in dims])
    a.offset = a.offset + base
    return a


def _build_nc():
    nc = bacc.Bacc(None, target_bir_lowering=False, debug=False)
    d_fpt = nc.dram_tensor("fpt", [101, 2240], E4, kind="ExternalInput")
    d_wt = nc.dram_tensor("wt", [101, 2560], E4, kind="ExternalInput")
    d_pb0 = nc.dram_tensor("pb0", [9, 768], E4, kind="ExternalInput")
    d_pb1 = nc.dram_tensor("pb1", [9, 768], E4, kind="ExternalInput")
    d_g0 = nc.dram_tensor("g0", [128, sum(STRIPS[0])], E4, kind="ExternalInput")
    d_g1 = nc.dram_tensor("g1", [128, sum(STRIPS[1])], E4, kind="ExternalInput")
    d_g2 = nc.dram_tensor("g2", [128, sum(STRIPS[2])], E4, kind="ExternalInput")
    d_wrt = nc.dram_tensor("wrt", [128, 512], BF16, kind="ExternalInput")
    d_bregt = nc.dram_tensor("bregt", [1, 512], BF16, kind="ExternalInput")
    d_out = nc.dram_tensor("out", [2, 512], F32, kind="ExternalOutput")

    with tile.TileContext(nc) as tc, ExitStack() as ctx:
        const = ctx.enter_context(tc.tile_pool(name="const", bufs=1))
        trash = ctx.enter_context(tc.tile_pool(name="trash", bufs=4))
        ps_enc = ctx.enter_context(tc.tile_pool(name="ps_enc", bufs=1, space="PSUM"))
        ps_g = ctx.enter_context(tc.tile_pool(name="ps_g", bufs=1, space="PSUM"))

        t_fpt = const.tile([101, 2240], E4, tag="fpt")
        t_wt = const.tile([101, 2560], E4, tag="wt")
        t_g = [const.tile([128, sum(STRIPS[ty])], E4, tag=f"g{ty}",
                          name=f"g{ty}") for ty in range(3)]
        t_P = [const.tile([128, 768], E4, tag=f"P{idx}", name=f"P{idx}")
               for idx in range(2)]
        t_wrt = const.tile([128, 512], BF16, tag="wrt")
        t_bregt = const.tile([1, 512], BF16, tag="bregt")
        t_ones2 = const.tile([1, 2], BF16, tag="ones2")
        t_zb = const.tile([128, 1], F32, tag="zb")
        t_part = const.tile([128, 16], F32, tag="part")
        t_m2 = const.tile([128, 2], BF16, tag="m2")
        t_t1 = const.tile([128, 2], F32, tag="t1")
        t_out = const.tile([2, 512], F32, tag="outT")

        nc.gpsimd.memset(t_zb[:], 0.0)
        nc.gpsimd.memset(t_ones2[:], 1.0)
        # prime the ACT spline-table load (~1.3us) during the head DMAs
        nc.scalar.activation(t_t1[0:1, 0:1], t_zb[0:1, 0:1],
                             mybir.ActivationFunctionType.Relu)

        nc.sync.dma_start(t_fpt[:, 0:1120], d_fpt[:, 0:1120])     # sample 0
        nc.sync.dma_start(t_wt[:], d_wt[:])
        nc.sync.dma_start(t_fpt[:, 1120:2240], d_fpt[:, 1120:2240])
        nc.sync.dma_start(t_P[0][119:128, :], d_pb0[:])
        nc.sync.dma_start(t_P[1][119:128, :], d_pb1[:])
        nc.sync.dma_start(t_g[0][:], d_g0[:])
        nc.sync.dma_start(t_g[1][:], d_g1[:])
        nc.sync.dma_start(t_g[2][:], d_g2[:])
        nc.sync.dma_start(t_wrt[:], d_wrt[:])
        nc.sync.dma_start(t_bregt[:], d_bregt[:])

        gt = ps_g.tile([128, 3072], F32, tag="gt")

        # whole-level encoders: out[h, t] blocks in gtile bank 5 tail
        for s in range(2):
            for idx in range(2):
                for w_ in range(2):
                    col = WCOL0 + s * 48 + (idx * 2 + w_) * 12
                    lhsT = _ap3(t_wt, 1536 + (idx * 2 + w_) * 256, 101, 128, 128)
                    rhs = _ap3(t_fpt, s * 1120 + idx * 560 + 512 + w_ * 24,
                               101, 12, 12)
                    nc.tensor.matmul(gt[:, col:col + 12], lhsT, rhs,
                                     start=True, stop=True, perf_mode=DR)
        for s in range(2):
            trw = trash.tile([128, 1536], BF16, tag="tr")
            nc.vector.tensor_scalar(trw[:, 0:48], gt[:, WCOL0 + s * 48:WCOL0 + s * 48 + 48],
                                    0.0, 0.0, mybir.AluOpType.max, mybir.AluOpType.add,
                                    accum_out=t_part[:, 8 + s:9 + s])

        for u, (s, idx) in enumerate([(0, 0), (0, 1), (1, 0), (1, 1)]):
            enc = ps_enc.tile([128, 1024], F32, tag="enc")
            fbase = s * 1120 + idx * 560
            rhs_w = _ap3(t_wt, idx * 768, 101, 384, 384)
            nc.tensor.matmul(enc[:, 128:512], _ap3(t_fpt, fbase, 101, 128, 128),
                             rhs_w, start=True, stop=True, perf_mode=DR)
            nc.tensor.matmul(enc[:, 512:896], _ap3(t_fpt, fbase + 256, 101, 128, 128),
                             rhs_w, start=True, stop=True, perf_mode=DR)
            # one 768-col copy: src (half, ty, h) -> dst ty-major kt-minor fp8
            src = _apn(enc, 128, 119, [(384, 2), (128, 3), (1, 128)])
            dst = _apn(t_P[idx], 0, 119, [(128, 2), (256, 3), (1, 128)])
            if COPY_ENG[u] == 'A':
                nc.scalar.copy(dst, src)
            else:
                nc.vector.tensor_copy(dst, src)
            for ty, a, b, gc in GCHUNKS:
                lhsT = _ap3(t_P[idx], ty * 256, 128, 128, 128)
                rhs = _ap3(t_g[ty], a, 128, STRIPS[ty][2], b - a)
                nc.tensor.matmul(gt[:, gc:gc + (b - a)], lhsT, rhs,
                                 start=True, stop=True, perf_mode=DR)
            for (ra, rb, eng) in RELU_CHUNKS:
                tr = trash.tile([128, 1536], BF16, tag="tr")
                pcol = u * 2 + (0 if ra == 0 else 1)
                if eng == 'A':
                    nc.scalar.activation(tr[:, 0:rb - ra], gt[:, ra:rb],
                                         mybir.ActivationFunctionType.Relu,
                                         bias=t_zb[:], accum_out=t_part[:, pcol:pcol + 1])
                else:
                    nc.vector.tensor_scalar(tr[:, 0:rb - ra], gt[:, ra:rb],
                                            0.0, 0.0, mybir.AluOpType.max,
                                            mybir.AluOpType.add,
                                            accum_out=t_part[:, pcol:pcol + 1])

        for s in range(2):
            nc.vector.tensor_reduce(t_t1[:, s:s + 1], t_part[:, 4 * s:4 * s + 4],
                                    mybir.AxisListType.X, mybir.AluOpType.add)
            nc.vector.scalar_tensor_tensor(t_m2[:, s:s + 1], t_part[:, 8 + s:9 + s],
                                           float(J), t_t1[:, s:s + 1],
                                           mybir.AluOpType.mult, mybir.AluOpType.add)

        rg = gt[0:2, 2048:2560]
        nc.tensor.matmul(rg, t_m2[:], t_wrt[:], start=True, stop=False)
        nc.tensor.matmul(rg, t_ones2[:], t_bregt[:], start=False, stop=True)
        nc.scalar.activation(t_out[:], rg, mybir.ActivationFunctionType.Relu,
                             bias=t_zb[0:2, :])
        nc.sync.dma_start(d_out[:], t_out[:])
    nc.compile()
    return nc


_NC = None


def _get_nc():
    global _NC
    if _NC is None:
        _NC = _build_nc()
    return _NC


def _run(inputs, **kw):
    nc = _get_nc()
    C = _host_constants(inputs)
    in_maps = []
    for c in range(8):
        m = dict(wt=C['wt'], pb0=C['pb0'], pb1=C['pb1'],
                 g0=C['g0'], g1=C['g1'], g2=C['g2'],
                 wrt=C['wrt'], bregt=C['bregt'])
        m['fpt'] = _host_fp(inputs, 2 * c)
        in_maps.append(m)
    res = run_bass_kernel_spmd(nc, in_maps, core_ids=list(range(8)), **kw)
    out = np.concatenate([r['out'] for r in res.results], 0)  # [16, 512]
    return out.reshape(16, 1, 512).astype(np.float32), res


def kernel(**inputs) -> np.ndarray:
    return _run(inputs)[0]


if __name__ == '__main__':
    import sys
    sys.path.insert(0, '/root/problem')
    import reference
    inputs = {k: np.asarray(v) for k, v in reference.setup_inputs().items()}
    expected = np.asarray(reference.reference(**inputs))
    actual = _emulate(inputs)
    err = np.abs(actual - expected).max() / np.abs(expected).max()
    print(f"emulator rel err: {err:.4e}")
